# revision 2
# baseline (speedup 1.0000x reference)
"""Trainium2 Bass kernel — int8 I/O edition.

Computation per (clip n, channel c):
  pooled[u]  = mean_{t,h,w} x[n,c,u,...]                (U=4 segments)
  z          = relu(BN(pooled @ W1^T))                  (tiny MLP, eval-mode BN)
  kern       = softmax(z @ W2^T)                        (K=3 taps)
  out[u]     = kern[0]*x[u-1] + kern[1]*x[u] + kern[2]*x[u+1]

Sharding: data-parallel, 1 clip per core.  HBM/fabric traffic is the
roofline, so both directions are 8-bit:
  - x quantized host-side to int8 with s = max|x|/127; loaded via SWDGE
    cast-DMA (int8 HBM -> fp16 SBUF, integer values exact in fp16).
  - out stored as int8.  PSUM = sum_j kern_j q_j is already in int8
    range (convex combo of |q|<=127), so the evac is a plain
    round-to-nearest copy and the host multiplies by the same s.
Per-core fabric: 12.85MB fp16-side loads + 6.4MB int8 stores ~= 44us.

Engine split (TRN2 1x errata; no fast modes survive accum/PSUM):
  - DVE pools via scalar_tensor_tensor(x_lo + x_hi, accum_out) — reads
    2 cols/cycle even at 1x (~0.57 ns/col) — plus the generator chains
    and one evac chunk per block.
  - ACT takes a small pool slice + most PSUM->int8 evacs (~0.93 ns/col).
  - GpSimd ONLY generates DMA descriptors: its tensor ops stall
    concurrent DVE ops on the shared SBUF port (measured 3.5us ADDs
    stretching DVE ops to the same span), so no pooling there.
  - x-load DGEs go first (wpack last): the SWDGE prep queue is ~8 deep
    and the 9th dma_start stalls until a slot frees.
Generator chains are per-block for blocks 0/1 (earliest possible first
blend -> PE stream start ~19us) and per-pair after (amortizes the
~200ns/op small-op overhead).
"""

import numpy as np

import concourse.bass as bass
import concourse.bacc as bacc
import concourse.tile as tile
from concourse import mybir
from concourse.bass_utils import run_bass_kernel_spmd

U = 4
C = 256
T, H, W = 8, 28, 28
THW = T * H * W            # 6272
D = 8
K = 3
EPS = 1e-5
N_CORES = 8
NB = 8
CS = 32

NPACK = D * U + K * D + D + D          # 72
# cpack_f32 cols: selp(32) | i4(4) | selt1(128) | selt_e(128) | selt_o(128)
NC32 = CS + U + 3 * 128

# DVE stt pool slice per block (input cols); ACT gets the rest.  Blocks
# 0/1 lean ACT (its queue is idle early); later blocks lean DVE-stt.
PDS = [5248, 5248, 5248, 5248, 5248, 5248, 6272, 6272]

# blend PSUM chunks; evac engine set per block in blend_ev
CHUNKS = [(0, 1024), (1024, 1024), (2048, 1024), (3072, 1024),
          (4096, 1024), (5120, 1024), (6144, 128)]
DVE_CH = {0: {5}, 1: {5}, 2: {5}, 3: {5}, 4: {4, 5}, 5: {4, 5},
          6: {4, 5}, 7: {4, 5}}
STORES = [(0, 2048), (2048, 2048), (4096, 2176)]

FP32 = mybir.dt.float32
FP16 = mybir.dt.float16
I8 = mybir.dt.int8

_nc_cache = None
last_results = None


def _bcast_ap(ap, parts=128):
    return bass.AP(tensor=ap.tensor, offset=ap.offset, ap=[[0, parts]] + list(ap.ap))


def _bcast_free(ap, n):
    """repeat a [P, F] AP n times along a new middle free dim."""
    return bass.AP(tensor=ap.tensor, offset=ap.offset,
                   ap=[list(ap.ap[0]), [0, n]] + list(ap.ap[1:]))


def _build_nc():
    nc = bacc.Bacc(None, target_bir_lowering=False)
    x_h = nc.declare_dram_parameter("x", [U, C, THW], I8, isOutput=False)
    wp_h = nc.declare_dram_parameter("wpack", [128, NPACK], FP32, isOutput=False)
    c32_h = nc.declare_dram_parameter("cpack32", [128, NC32], FP32, isOutput=False)
    c16_h = nc.declare_dram_parameter("cpack16", [128, K * 128], FP16, isOutput=False)
    out_h = nc.declare_dram_parameter("out", [U, C, THW], I8, isOutput=True)

    xg = x_h[:].rearrange("u (b cs) f -> b cs u f", b=NB)   # [8, 32, 4, THW]
    og = out_h[:].rearrange("u (b cs) f -> b cs u f", b=NB)

    AX = mybir.AxisListType
    OP = mybir.AluOpType
    AF = mybir.ActivationFunctionType
    PSUM = bass.MemorySpace.PSUM

    with tile.TileContext(nc) as tc:
        with (
            tc.tile_pool(name="xp", bufs=8) as xp,
            tc.tile_pool(name="outp", bufs=3) as outp,
            tc.tile_pool(name="small", bufs=1) as small,
            tc.tile_pool(name="wp", bufs=2) as wp,
            tc.tile_pool(name="mlp", bufs=2) as mlp,
            tc.tile_pool(name="pbig", bufs=3, space=PSUM) as pbig,
            tc.tile_pool(name="psmall", bufs=1, space=PSUM) as psmall,
        ):
            # ---- x loads: int8 cast-DMA on the SWDGE queue; consts on
            # the sync HWDGE ring (stores join it later) ----
            xbs = []
            for b in range(NB):
                xb = xp.tile([128, THW], FP16, tag="xblk", name=f"xb{b}")
                nc.gpsimd.dma_start(out=xb, in_=xg[b])
                xbs.append(xb)
            wpk = small.tile([128, NPACK], FP32)
            nc.sync.dma_start(out=wpk, in_=wp_h[:])
            cp32 = small.tile([128, NC32], FP32)
            nc.sync.dma_start(out=cp32, in_=c32_h[:])
            cp16 = small.tile([128, K * 128], FP16)
            nc.sync.dma_start(out=cp16, in_=c16_h[:])

            w1sb = wpk[:, 0:D * U].rearrange("p (d u) -> p d u", d=D)
            w2sb = wpk[:, D * U:D * U + K * D].rearrange("p (k d) -> p k d", k=K)
            o_t = wpk[:, D * U + K * D + D:NPACK]
            selp = cp32[:, 0:CS]
            i4 = cp32[:, CS:CS + U]
            selt1 = cp32[:, CS + U:CS + U + 128]
            selt_e = cp32[:, CS + U + 128:CS + U + 2 * 128]
            selt_o = cp32[:, CS + U + 2 * 128:NC32]

            def pat(j):
                return cp16[:, j * 128:(j + 1) * 128]

            # pooled partials: [128, b, 2] = (dve stt, act) per block
            pooled2 = small.tile([128, NB, 2], FP32)
            nc.vector.memset(pooled2, 0.0)
            scratch = small.tile([128, 3136], FP16)      # stt dump

            def pool_b(b):
                pd = PDS[b]
                hd = pd // 2
                nc.vector.scalar_tensor_tensor(
                    out=scratch[:, 0:hd], in0=xbs[b][:, 0:hd], scalar=1.0,
                    in1=xbs[b][:, hd:pd], op0=OP.mult, op1=OP.add,
                    accum_out=pooled2[:, b, 0:1])
                if pd < THW:
                    nc.scalar.activation(
                        out=xbs[b][:, pd:THW], in_=xbs[b][:, pd:THW],
                        func=AF.Copy, accum_out=pooled2[:, b, 1:2])

            def chain_pre(blist):
                """pool finish -> inb (DVE only)."""
                nb = len(blist)
                b0 = blist[0]
                pf = mlp.tile([128, nb], FP32, tag="pf", name=f"pf{b0}")
                nc.vector.reduce_sum(out=pf, in_=pooled2[:, b0:b0 + nb, :],
                                     axis=AX.X)
                inb = mlp.tile([128, nb, U], FP32, tag="inb", name=f"inb{b0}")
                i4b = bass.AP(tensor=i4.tensor, offset=i4.offset,
                              ap=[list(i4.ap[0]), [0, nb]] + list(i4.ap[1:]))
                pfb = bass.AP(tensor=pf.tensor, offset=pf.offset,
                              ap=[list(pf.ap[0])] + list(pf.ap[1:]) + [[0, U]])
                nc.vector.tensor_tensor(out=inb, in0=i4b, in1=pfb, op=OP.mult)
                return inb

            def chain_ppmm(blist, inb):
                """channel-major transform matmul (PE) — emitted at a
                safe slot in the PE queue by the caller."""
                nb = len(blist)
                b0 = blist[0]
                pp = psmall.tile([128, nb * U], FP32, tag="pp", name=f"pp{b0}")
                nc.tensor.matmul(out=pp[0:CS, :], lhsT=selp,
                                 rhs=inb.rearrange("p b u -> p (b u)"),
                                 start=True, stop=True)
                return pp

            def chain_post(blist, pp):
                """pooledP copies + MLP -> kern (DVE + ACT exp)."""
                nb = len(blist)
                P = CS * nb
                b0 = blist[0]
                pooledP = mlp.tile([P, U], FP32, tag="pooledP", name=f"pP{b0}")
                for j in range(nb):
                    nc.scalar.copy(out=pooledP[j * CS:(j + 1) * CS, :],
                                   in_=pp[0:CS, j * U:(j + 1) * U])
                zp = mlp.tile([P, D, U], FP32, tag="zp", name=f"zp{b0}")
                nc.vector.tensor_tensor(out=zp, in0=w1sb[0:P],
                                        in1=_bcast_free(pooledP, D), op=OP.mult)
                z = mlp.tile([P, D], FP32, tag="z", name=f"z{b0}")
                nc.vector.reduce_sum(out=z, in_=zp, axis=AX.X)
                nc.vector.tensor_add(out=z, in0=z, in1=o_t[0:P])
                nc.vector.tensor_scalar_max(out=z, in0=z, scalar1=0.0)
                lp = mlp.tile([P, K, D], FP32, tag="lp", name=f"lp{b0}")
                nc.vector.tensor_tensor(out=lp, in0=w2sb[0:P],
                                        in1=_bcast_free(z, K), op=OP.mult)
                logit = mlp.tile([P, K], FP32, tag="logit", name=f"lg{b0}")
                nc.vector.reduce_sum(out=logit, in_=lp, axis=AX.X)
                ssum = mlp.tile([P, 1], FP32, tag="ssum", name=f"ss{b0}")
                nc.scalar.activation(out=logit, in_=logit, func=AF.Exp,
                                     accum_out=ssum)
                nc.vector.reciprocal(out=ssum, in_=ssum)
                kern = mlp.tile([P, K], FP32, tag="kern", name=f"kern{b0}")
                nc.vector.tensor_scalar_mul(out=kern, in0=logit,
                                            scalar1=ssum[:, 0:1])
                return kern

            def chain_kpmm(b, kern, sel, selP):
                """selector matmul kern -> kp[(cs,u'), k] (PE)."""
                kp = psmall.tile([128, U], FP32, tag="kp", name=f"kp{b}")
                nc.tensor.matmul(out=kp[:, 0:K], lhsT=sel[0:selP], rhs=kern,
                                 start=True, stop=True)
                return kp

            def wbuild(b, kp):
                """W_b[(cs,u'),(cs,u)] = kern[cs(b), u'-u+1] as fp16 SBUF;
                the per-partition scalars come straight from PSUM kp."""
                wt = wp.tile([128, 128], FP16, tag="W", name=f"W{b}")
                w1t = wp.tile([128, 128], FP16, tag="Wt1", name=f"Wa{b}")
                w2t = wp.tile([128, 128], FP16, tag="Wt2", name=f"Wb{b}")
                nc.vector.tensor_scalar_mul(out=wt, in0=pat(0), scalar1=kp[:, 0:1])
                nc.vector.tensor_scalar_mul(out=w1t, in0=pat(1), scalar1=kp[:, 1:2])
                nc.vector.tensor_scalar_mul(out=w2t, in0=pat(2), scalar1=kp[:, 2:3])
                nc.vector.tensor_tensor(out=wt, in0=wt, in1=w1t, op=OP.add)
                nc.vector.tensor_tensor(out=wt, in0=wt, in1=w2t, op=OP.add)
                return wt

            pts = {}

            def blend_mm(b, wt, at_c2=None, at_end=None):
                """PE matmuls -> PSUM fp32 chunks; hooks emit the tiny
                chain matmuls at safe PE-queue slots."""
                for ci, (off, ln) in enumerate(CHUNKS):
                    pt = pbig.tile([128, 1024], FP32, tag="pb",
                                   name=f"pb{b}_{ci}")
                    pts[(b, ci)] = pt
                    for s in range(0, ln, 512):
                        w = min(512, ln - s)
                        nc.tensor.matmul(
                            out=pt[:, s:s + w], lhsT=wt,
                            rhs=xbs[b][:, off + s:off + s + w],
                            start=True, stop=True)
                    if ci == 2 and at_c2 is not None:
                        at_c2()
                if at_end is not None:
                    at_end()

            def blend_ev(b):
                """PSUM -> int8 SBUF (round-to-nearest) -> store."""
                osb = outp.tile([128, THW], I8, tag="osb", name=f"osb{b}")
                for ci, (off, ln) in enumerate(CHUNKS):
                    pt = pts[(b, ci)]
                    if ci in DVE_CH[b]:
                        nc.vector.tensor_scalar(
                            out=osb[:, off:off + ln], in0=pt[:, 0:ln],
                            scalar1=1.0, scalar2=None, op0=OP.mult)
                    else:
                        nc.scalar.activation(
                            out=osb[:, off:off + ln], in_=pt[:, 0:ln],
                            func=AF.Copy)
                for off, ln in STORES:
                    nc.sync.dma_start(out=og[b][:, :, off:off + ln],
                                      in_=osb[:, off:off + ln])

            # ---- pipeline: chain matmuls ride inside the previous
            # blend's PE emission; evacs trail; blocks 6/7 run single
            # chains so the post-last-load tail is short ----
            wts = [None] * NB
            holder = {}

            pool_b(0)
            inb0 = chain_pre([0])
            pp0 = chain_ppmm([0], inb0)
            k0 = chain_post([0], pp0)
            kp0 = chain_kpmm(0, k0, selt1, CS)
            wts[0] = wbuild(0, kp0)

            pool_b(1)
            inb1 = chain_pre([1])

            def mm0_c2():
                holder['pp1'] = chain_ppmm([1], inb1)
            blend_mm(0, wts[0], at_c2=mm0_c2)
            k1 = chain_post([1], holder['pp1'])
            kp1 = chain_kpmm(1, k1, selt1, CS)
            wts[1] = wbuild(1, kp1)
            blend_ev(0)

            pool_b(2)
            pool_b(3)
            inb23 = chain_pre([2, 3])

            def mm1_c2():
                holder['pp23'] = chain_ppmm([2, 3], inb23)
            blend_mm(1, wts[1], at_c2=mm1_c2)
            k23 = chain_post([2, 3], holder['pp23'])
            kp2 = chain_kpmm(2, k23, selt_e, 2 * CS)
            kp3 = chain_kpmm(3, k23, selt_o, 2 * CS)
            wts[2] = wbuild(2, kp2)
            wts[3] = wbuild(3, kp3)
            blend_ev(1)

            pool_b(4)
            pool_b(5)
            inb45 = chain_pre([4, 5])

            def mm2_c2():
                holder['pp45'] = chain_ppmm([4, 5], inb45)
            blend_mm(2, wts[2], at_c2=mm2_c2)
            k45 = chain_post([4, 5], holder['pp45'])
            kp4 = chain_kpmm(4, k45, selt_e, 2 * CS)
            kp5 = chain_kpmm(5, k45, selt_o, 2 * CS)
            wts[4] = wbuild(4, kp4)
            wts[5] = wbuild(5, kp5)
            blend_ev(2)

            pool_b(6)
            inb6 = chain_pre([6])

            def mm3_end():
                holder['pp6'] = chain_ppmm([6], inb6)
            blend_mm(3, wts[3], at_end=mm3_end)
            k6 = chain_post([6], holder['pp6'])
            blend_ev(3)

            pool_b(7)
            inb7 = chain_pre([7])

            def mm4_c2():
                holder['kp6'] = chain_kpmm(6, k6, selt1, CS)

            def mm4_end():
                holder['pp7'] = chain_ppmm([7], inb7)
            blend_mm(4, wts[4], at_c2=mm4_c2, at_end=mm4_end)
            wts[6] = wbuild(6, holder['kp6'])
            k7 = chain_post([7], holder['pp7'])
            blend_ev(4)

            def mm5_c2():
                holder['kp7'] = chain_kpmm(7, k7, selt1, CS)
            blend_mm(5, wts[5], at_c2=mm5_c2)
            wts[7] = wbuild(7, holder['kp7'])
            blend_ev(5)
            blend_mm(6, wts[6])
            blend_ev(6)
            blend_mm(7, wts[7])
            blend_ev(7)
    nc.finalize()
    return nc


def _get_nc():
    global _nc_cache
    if _nc_cache is None:
        _nc_cache = _build_nc()
    return _nc_cache


def _pack_small(W1, bn_gamma, bn_beta, bn_mean, bn_var, W2, s_in):
    W1 = np.asarray(W1, np.float32)
    W2 = np.asarray(W2, np.float32)
    gam = np.asarray(bn_gamma, np.float32)
    bet = np.asarray(bn_beta, np.float32)
    mea = np.asarray(bn_mean, np.float32)
    var = np.asarray(bn_var, np.float32)
    sc = (gam / np.sqrt(var + np.float32(EPS))).astype(np.float32)
    t = (bet - mea * sc).astype(np.float32)
    w1s = (W1 * sc[:, None] * np.float32(s_in / THW)).astype(np.float32)
    pk = np.concatenate(
        [w1s.reshape(-1), W2.reshape(-1), sc, t]).astype(np.float32)
    return np.ascontiguousarray(np.broadcast_to(pk, (128, NPACK)))


def _pack_consts():
    p = np.arange(128)
    selp = np.zeros((128, CS), np.float32)
    selp[p, p // 4] = 1.0
    i4 = np.zeros((128, U), np.float32)
    i4[p, p % 4] = 1.0
    selt1 = np.zeros((128, 128), np.float32)   # kern[cs,k] -> kp[(cs,u'),k]
    selt1[p // 4, p] = 1.0
    selt_e = np.zeros((128, 128), np.float32)  # pair chains: even block
    selt_e[p // 4, p] = 1.0                    # rows 0-31 (c64 < 32)
    selt_o = np.zeros((128, 128), np.float32)  # odd block: rows 32-63
    selt_o[32 + p // 4, p] = 1.0
    cp32 = np.concatenate([selp, i4, selt1, selt_e, selt_o], axis=1)
    pats = np.zeros((K, 128, 128), np.float16)
    for j in range(K):
        u = (p % 4) + 1 - j
        valid = (0 <= u) & (u < 4)
        pats[j, p[valid], (p // 4 * 4 + u)[valid]] = 1.0
    cp16 = pats.transpose(1, 0, 2).reshape(128, K * 128).astype(np.float16)
    return np.ascontiguousarray(cp32), np.ascontiguousarray(cp16)


def _ensure_hook_stub():
    import sys
    import types
    try:
        import antenv.axon_hooks  # noqa: F401
    except ImportError:
        mod = types.ModuleType("antenv.axon_hooks")
        mod.get_axon_ntff_profile_hook = lambda: None
        mod.set_axon_ntff_profile_hook = lambda h: None
        sys.modules["antenv.axon_hooks"] = mod


def kernel(x, W1, bn_gamma, bn_beta, bn_mean, bn_var, W2):
    global last_results
    _ensure_hook_stub()
    nc = _get_nc()
    x = np.ascontiguousarray(np.asarray(x, dtype=np.float32)).reshape(
        N_CORES, U, C, THW)
    s_in = float(np.abs(x).max()) / 127.0
    xq = np.clip(np.rint(x * np.float32(1.0 / s_in)), -127, 127).astype(np.int8)
    wpack = _pack_small(W1, bn_gamma, bn_beta, bn_mean, bn_var, W2, s_in)
    cp32, cp16 = _pack_consts()
    in_maps = [
        {"x": xq[i], "wpack": wpack, "cpack32": cp32, "cpack16": cp16}
        for i in range(N_CORES)
    ]
    last_results = run_bass_kernel_spmd(nc, in_maps, list(range(N_CORES)))
    out = np.stack([last_results.results[i]["out"] for i in range(N_CORES)])
    return (out.astype(np.float32) * np.float32(s_in)).reshape(
        N_CORES * U, C, T, H, W)
